# revision 1
# baseline (speedup 1.0000x reference)
"""Trainium2 Bass kernel for a post-LN transformer decoder layer.

Sharding: 8 cores = 4 batches x 2 token-halves. Core c=(b,j) handles batch b
and 2 query quarter-blocks (j=0: global rows [0:512)+[1536:2048),
j=1: [512:1536)) -- interleaved quarters balance the causal triangle.
All matmul operands bf16 (fp32 PSUM accumulate), residual/LN path fp32.
Everything is computed in transposed layout ([d_model, tokens]) so matmul
contraction stays on the partition axis with zero on-chip transposes; host
transposes inputs/outputs instead.

Cross-attention needs V2 = x1 @ wv2 for the FULL batch sequence, so each
pair of cores AllGathers its half (the only collective in the kernel).

The program must be identical on all 8 cores (single SPMD NEFF), so the
causal structure is padded to a uniform (8, 16) k-tile schedule per query
slot and all causal masking comes from per-core host-computed mask data.
"""

import sys

sys.path.insert(0, "/opt/trn_rl_repo")

import numpy as np
import ml_dtypes

import concourse.bass as bass
import concourse.tile as tile
from concourse import mybir
from concourse.bass_utils import run_bass_kernel_spmd

BF16 = mybir.dt.bfloat16
F32 = mybir.dt.float32
AF = mybir.ActivationFunctionType

D = 1024       # d_model
DFF = 4096
B, S = 4, 2048
NCORES = 8
QL = 1024      # local query rows per core
EPS = 1e-6
MT = 8         # d_model 128-tiles
FT = 32        # d_ff 128-tiles
SCALE = 0.125  # 1/sqrt(head_dim)
SLOT_NKT = (8, 16)  # uniform k-tile count per query slot; last 8 are masked

# AllGather is per core-pair; pair-local rank j owns quarters (Q0,Q3) for j=0
# and (Q1,Q2) for j=1.  v2all row base for global k-tile t is
# V2_ROW0[t//4] + (t%4)*128.
V2_ROW0 = [0, 1024, 1536, 512]


def legalize_waits(nc, max_waits=1):
    """This walrus build accepts at most one sync-wait per instruction.
    Hoist excess waits onto same-engine NoOps inserted just before."""
    nid = 0
    for fn in nc.m.functions:
        for bb in fn.blocks:
            new = []
            changed = False
            for inst in bb.instructions:
                si = inst.sync_info
                if si is not None and si.on_wait and len(si.on_wait) > max_waits:
                    waits = list(si.on_wait)
                    for w in waits[:-max_waits]:
                        nid += 1
                        nop = mybir.InstNoOp(name=f"I-waitfix-{nid}", ins=[], outs=[])
                        nop.engine = inst.engine
                        nop.sync_info = mybir.SyncInfo(on_wait=[w], on_update=[])
                        new.append(nop)
                    inst.sync_info = mybir.SyncInfo(
                        on_wait=waits[-max_waits:], on_update=list(si.on_update)
                    )
                    changed = True
                new.append(inst)
            if changed:
                bb.instructions = new


def build_nc():
    nc = bass.Bass(num_devices=NCORES)

    xT = nc.dram_tensor("xT", [D, S], BF16, kind="ExternalInput")
    xTq = nc.dram_tensor("xTq", [D, QL], BF16, kind="ExternalInput")
    xres = nc.dram_tensor("xres", [D, QL], F32, kind="ExternalInput")
    encT = nc.dram_tensor("encT", [D, S], BF16, kind="ExternalInput")
    encTq = nc.dram_tensor("encTq", [D, QL], BF16, kind="ExternalInput")
    wd = {}
    for name in ("wq1", "wk1", "wv1", "wo1", "wq2", "wk2", "wv2", "wo2"):
        wd[name] = nc.dram_tensor(name, [D, D], BF16, kind="ExternalInput")
    wd["wf1"] = nc.dram_tensor("wf1", [D, DFF], BF16, kind="ExternalInput")
    wd["wf2"] = nc.dram_tensor("wf2", [DFF, D], BF16, kind="ExternalInput")
    bias_specs = (
        ("cq1", 8), ("ck1", 8), ("co1", 8), ("cq2", 8), ("ck2", 8), ("co2", 8),
        ("cf1", 32), ("cf2", 8),
        ("g1", 8), ("be1", 8), ("g2", 8), ("be2", 8), ("g3", 8), ("be3", 8),
    )
    bcd = {}
    for name, k in bias_specs:
        bcd[name] = nc.dram_tensor(name, [128, k], F32, kind="ExternalInput")
    rv1 = nc.dram_tensor("rv1", [1, D], BF16, kind="ExternalInput")
    rv2 = nc.dram_tensor("rv2", [1, D], BF16, kind="ExternalInput")
    bigmask = nc.dram_tensor("bigmask", [2, 8, 128, 512], BF16, kind="ExternalInput")
    outT = nc.dram_tensor("outT", [D, QL], F32, kind="ExternalOutput")
    v2loc = nc.dram_tensor("v2loc", [QL, D], BF16)
    v2all = nc.dram_tensor("v2all", [2 * QL, D], BF16)

    def wslices(name):
        return wd[name].ap().rearrange("(mt p) d -> p mt d", p=128)

    with tile.TileContext(nc) as tc:
        _cms = {}

        def open_pool(**kw):
            cm = tc.tile_pool(**kw)
            _cms[kw["name"]] = cm
            return cm.__enter__()

        def close_pool(pool_name):
            _cms.pop(pool_name).__exit__(None, None, None)

        # long-lived pools (whole kernel)
        const = open_pool(name="const", bufs=1)
        wpool = open_pool(name="wpool", bufs=2)
        hpool = open_pool(name="hpool", bufs=2)
        epool = open_pool(name="epool", bufs=4)
        sp1 = open_pool(name="sp1", bufs=1)    # LN row scratch
        sp2 = open_pool(name="sp2", bufs=2)    # attention normalize scratch
        lnz = open_pool(name="lnz", bufs=1)    # z accumulator
        lnsc = open_pool(name="lnsc", bufs=2)  # per-mt LN scratch
        psp = open_pool(name="psp", bufs=2, space="PSUM")

        # ---- constants ----
        bc = {}
        for name, k in bias_specs:
            t = const.tile([128, k], F32, tag=name)
            nc.sync.dma_start(out=t, in_=bcd[name][:, :])
            bc[name] = t
        rv1_sb = const.tile([1, D], BF16, tag="rv1")
        nc.sync.dma_start(out=rv1_sb, in_=rv1[:, :])
        rv2_sb = const.tile([1, D], BF16, tag="rv2")
        nc.sync.dma_start(out=rv2_sb, in_=rv2[:, :])
        ones_row = const.tile([1, 512], BF16, tag="ones_row")
        nc.vector.memset(ones_row, 1.0)
        ones_col = const.tile([128, 1], BF16, tag="ones_col")
        nc.vector.memset(ones_col, 1.0)
        epsr = const.tile([1, 1], F32, tag="epsr")
        nc.vector.memset(epsr, EPS)

        def ln_block(z, gname, bename, xout, xbfout, qb):
            """LayerNorm over the partition(d_model) axis of z [128, MT, 512]
            f32.  Writes xout[:, :, qb*512:+512] f32 (+ optional bf16 copy)."""
            psum_s = psp.tile([1, 512], F32, tag="u")
            psum_q = psp.tile([1, 512], F32, tag="u")
            for mt in range(MT):
                zbf = lnsc.tile([128, 512], BF16, tag="zbf")
                nc.scalar.activation(zbf[:, :], z[:, mt, :], AF.Copy)
                zsq = lnsc.tile([128, 512], BF16, tag="zsq")
                nc.vector.tensor_mul(zsq[:, :], zbf[:, :], zbf[:, :])
                nc.tensor.matmul(psum_s[:, :], ones_col[:, :], zbf[:, :],
                                 start=(mt == 0), stop=(mt == MT - 1))
                nc.tensor.matmul(psum_q[:, :], ones_col[:, :], zsq[:, :],
                                 start=(mt == 0), stop=(mt == MT - 1))
            mu = sp1.tile([1, 512], F32, tag="ln_mu")
            nc.scalar.activation(mu[:, :], psum_s[:, :], AF.Copy, scale=1.0 / D)
            t = sp1.tile([1, 512], F32, tag="ln_t")
            nc.scalar.activation(t[:, :], psum_q[:, :], AF.Copy, scale=1.0 / D)
            musq = sp1.tile([1, 512], F32, tag="ln_musq")
            nc.vector.tensor_mul(musq[:, :], mu[:, :], mu[:, :])
            nc.vector.tensor_sub(t[:, :], t[:, :], musq[:, :])
            nc.scalar.activation(t[:, :], t[:, :], AF.Sqrt, bias=epsr[:, :])
            rstd = sp1.tile([1, 512], F32, tag="ln_rstd")
            nc.vector.reciprocal(rstd[:, :], t[:, :])
            mubf = sp1.tile([1, 512], BF16, tag="ln_mubf")
            nc.scalar.activation(mubf[:, :], mu[:, :], AF.Copy)
            rsbf = sp1.tile([1, 512], BF16, tag="ln_rsbf")
            nc.scalar.activation(rsbf[:, :], rstd[:, :], AF.Copy)
            mu_b = psp.tile([128, 512], F32, tag="s")
            nc.tensor.matmul(mu_b[:, :], ones_row[:, 0:128], mubf[:, :],
                             start=True, stop=True)
            rs_b = psp.tile([128, 512], F32, tag="s")
            nc.tensor.matmul(rs_b[:, :], ones_row[:, 0:128], rsbf[:, :],
                             start=True, stop=True)
            qs = slice(qb * 512, qb * 512 + 512)
            g = bc[gname]
            be = bc[bename]
            for mt in range(MT):
                tmp = lnsc.tile([128, 512], F32, tag="lntmp")
                nc.vector.tensor_sub(tmp[:, :], z[:, mt, :], mu_b[:, :])
                nc.vector.tensor_mul(tmp[:, :], tmp[:, :], rs_b[:, :])
                nc.vector.tensor_scalar(
                    xout[:, mt, qs], tmp[:, :],
                    g[:, mt:mt + 1], be[:, mt:mt + 1],
                    op0=mybir.AluOpType.mult, op1=mybir.AluOpType.add,
                )
                if xbfout is not None:
                    nc.scalar.activation(xbfout[:, mt, qs], xout[:, mt, qs], AF.Copy)

        def attention(KTh, VH, QTh, attn_out, maskt, hp):
            """One head-pair of attention in transposed layout.
            KTh [128, S] bf16, VH [128, 16, 2, 65] bf16, QTh [128, QL] bf16.
            Writes attn_out[:, hp, :] (bf16), softmax-normalized."""
            # The two heads of the pair are interleaved per k-tile so the PE
            # always has the other head's score matmul to run while ACT
            # computes this head's exp (otherwise PE stalls every k-tile).
            for qb in range(2):
                nkt = SLOT_NKT[qb]
                qs = slice(qb * 512, qb * 512 + 512)
                us = []
                for _h in range(2):
                    u_t = psp.tile([65, 512], F32, tag="u")
                    us.append(u_t)
                for kt in range(nkt):
                    es = []
                    for h in range(2):
                        hs = slice(h * 64, h * 64 + 64)
                        s_ps = psp.tile([128, 512], F32, tag="s")
                        nc.tensor.matmul(
                            s_ps[:, :],
                            KTh[hs, kt * 128:kt * 128 + 128],
                            QTh[hs, qs],
                            start=True, stop=True,
                        )
                        e = epool.tile([128, 512], BF16, tag="e")
                        nc.scalar.activation(e[:, :], s_ps[:, :], AF.Exp,
                                             scale=SCALE)
                        if maskt is not None and kt >= nkt - 8:
                            ki = kt - (nkt - 8)
                            nc.vector.tensor_mul(e[:, :], e[:, :],
                                                 maskt[:, qb, ki, :])
                        es.append(e)
                    for h in range(2):
                        nc.tensor.matmul(
                            us[h][:, :], VH[:, kt, h, :], es[h][:, :],
                            start=(kt == 0), stop=(kt == nkt - 1),
                        )
                for h in range(2):
                    hs = slice(h * 64, h * 64 + 64)
                    u_sb = sp2.tile([65, 512], F32, tag="u_sb")
                    nc.vector.tensor_copy(u_sb[:, :], us[h][:, :])
                    rec = sp2.tile([1, 512], F32, tag="rec")
                    nc.vector.reciprocal(rec[:, :], u_sb[64:65, :])
                    recbf = sp2.tile([1, 512], BF16, tag="recbf")
                    nc.scalar.activation(recbf[:, :], rec[:, :], AF.Copy)
                    rb = psp.tile([64, 512], F32, tag="rb")
                    nc.tensor.matmul(rb[:, :], ones_row[:, 0:64], recbf[:, :],
                                     start=True, stop=True)
                    nc.vector.tensor_mul(
                        attn_out[hs, hp, qs], u_sb[0:64, :], rb[:, :]
                    )

        # ================= PHASE A: self-attention =================
        pA = open_pool(name="pA", bufs=1)                    # xT/xTq/mask
        pRES = open_pool(name="pRES", bufs=1, side="right")  # fp32 residual
        pAT = open_pool(name="pAT", bufs=1, side="right")    # attn1

        xTs = pA.tile([128, MT, S], BF16, tag="xTs")
        for mt in range(MT):
            nc.sync.dma_start(
                out=xTs[:, mt, :],
                in_=xT.ap().rearrange("(mt p) s -> p mt s", p=128)[:, mt, :],
            )
        xqs = pA.tile([128, MT, QL], BF16, tag="xqs")
        for mt in range(MT):
            nc.sync.dma_start(
                out=xqs[:, mt, :],
                in_=xTq.ap().rearrange("(mt p) s -> p mt s", p=128)[:, mt, :],
            )
        maskt = pA.tile([128, 2, 8, 512], BF16, tag="maskt")
        nc.sync.dma_start(
            out=maskt, in_=bigmask.ap().rearrange("sl ki p j -> p sl ki j")
        )
        xres_t = pRES.tile([128, MT, QL], F32, tag="xres_t")
        for mt in range(MT):
            nc.sync.dma_start(
                out=xres_t[:, mt, :],
                in_=xres.ap().rearrange("(mt p) s -> p mt s", p=128)[:, mt, :],
            )
        attn1 = pAT.tile([128, MT, QL], BF16, tag="attn1")

        for hp in range(MT):
            ds = slice(hp * 128, hp * 128 + 128)
            wq1s = wpool.tile([128, MT, 128], BF16, tag="wq1s")
            wk1s = wpool.tile([128, MT, 128], BF16, tag="wk1s")
            wv1s = wpool.tile([128, MT, 128], BF16, tag="wv1s")
            for nm, t in (("wq1", wq1s), ("wk1", wk1s), ("wv1", wv1s)):
                nc.sync.dma_start(out=t, in_=wslices(nm)[:, :, ds])
            KTh = hpool.tile([128, S], BF16, tag="KTh")
            for sb in range(4):
                ss = slice(sb * 512, sb * 512 + 512)
                pp = psp.tile([128, 512], F32, tag="pp")
                for mt in range(MT):
                    nc.tensor.matmul(pp[:, :], wk1s[:, mt, :], xTs[:, mt, ss],
                                     start=(mt == 0), stop=(mt == MT - 1))
                nc.scalar.activation(KTh[:, ss], pp[:, :], AF.Identity,
                                     bias=bc["ck1"][:, hp:hp + 1])
            QTh = hpool.tile([128, QL], BF16, tag="QTh")
            for qb in range(2):
                qs = slice(qb * 512, qb * 512 + 512)
                pp = psp.tile([128, 512], F32, tag="pp")
                for mt in range(MT):
                    nc.tensor.matmul(pp[:, :], wq1s[:, mt, :], xqs[:, mt, qs],
                                     start=(mt == 0), stop=(mt == MT - 1))
                nc.scalar.activation(QTh[:, qs], pp[:, :], AF.Identity,
                                     bias=bc["cq1"][:, hp:hp + 1])
            VH = hpool.tile([128, 16, 2, 65], BF16, tag="VH")
            nc.vector.memset(VH[:, :, :, 64:65], 1.0)
            for st in range(16):
                ts_ = slice(st * 128, st * 128 + 128)
                pp = psp.tile([128, 128], F32, tag="pp")
                for mt in range(MT):
                    nc.tensor.matmul(pp[:, :], xTs[:, mt, ts_], wv1s[:, mt, :],
                                     start=(mt == 0), stop=False)
                nc.tensor.matmul(pp[:, :], ones_row[:, 0:128], rv1_sb[:, ds],
                                 start=False, stop=True)
                nc.vector.tensor_copy(
                    VH[:, st, :, 0:64],
                    pp[:, :].rearrange("p (a b) -> p a b", a=2),
                )
            attention(KTh, VH, QTh, attn1, maskt, hp)

        close_pool("pA")  # free xT/xTq/mask

        # ---- out-proj 1 + residual + LN1 -> x1 ----
        pX1 = open_pool(name="pX1", bufs=1)
        pX1B = open_pool(name="pX1B", bufs=1)
        x1 = pX1.tile([128, MT, QL], F32, tag="x1")
        x1bf = pX1B.tile([128, MT, QL], BF16, tag="x1bf")
        for qb in range(2):
            qs = slice(qb * 512, qb * 512 + 512)
            z1 = lnz.tile([128, MT, 512], F32, tag="z1")
            for nt in range(MT):
                wo1s = wpool.tile([128, MT, 128], BF16, tag="wo1s")
                nc.sync.dma_start(
                    out=wo1s,
                    in_=wslices("wo1")[:, :, nt * 128:nt * 128 + 128],
                )
                pp = psp.tile([128, 512], F32, tag="pp")
                for dt in range(MT):
                    nc.tensor.matmul(pp[:, :], wo1s[:, dt, :], attn1[:, dt, qs],
                                     start=(dt == 0), stop=(dt == MT - 1))
                t1 = lnsc.tile([128, 512], F32, tag="lntmp")
                nc.scalar.activation(t1[:, :], pp[:, :], AF.Identity,
                                     bias=bc["co1"][:, nt:nt + 1])
                nc.vector.tensor_add(z1[:, nt, :], t1[:, :], xres_t[:, nt, qs])
            ln_block(z1, "g1", "be1", x1, x1bf, qb)

        close_pool("pAT")   # free attn1 (right stack top)
        close_pool("pRES")  # free xres

        # ================= V2 projection + AllGather =================
        pV2 = open_pool(name="pV2", bufs=1)
        wv2s = pV2.tile([128, MT, D], BF16, tag="wv2s")
        nc.sync.dma_start(out=wv2s, in_=wslices("wv2"))
        v2sb = pV2.tile([128, MT, D], BF16, tag="v2sb")
        for st in range(MT):
            ss = slice(st * 128, st * 128 + 128)
            for db in range(2):
                dsl = slice(db * 512, db * 512 + 512)
                pp = psp.tile([128, 512], F32, tag="pp")
                for mt in range(MT):
                    nc.tensor.matmul(pp[:, :], x1bf[:, mt, ss], wv2s[:, mt, dsl],
                                     start=(mt == 0), stop=False)
                nc.tensor.matmul(pp[:, :], ones_row[:, 0:128], rv2_sb[:, dsl],
                                 start=False, stop=True)
                nc.vector.tensor_copy(v2sb[:, st, dsl], pp[:, :])
        nc.sync.dma_start(
            out=v2loc.ap().rearrange("(st p) d -> p st d", p=128), in_=v2sb
        )
        nc.gpsimd.collective_compute(
            "AllGather",
            mybir.AluOpType.bypass,
            replica_groups=[[2 * p, 2 * p + 1] for p in range(4)],
            ins=[v2loc[:, :]],
            outs=[v2all[:, :]],
        )
        close_pool("pV2")
        close_pool("pX1B")  # x1bf only needed for the V2 projection

        # ================= PHASE B: cross-attention =================
        pAT2 = open_pool(name="pAT2", bufs=1)
        pENCS = open_pool(name="pENCS", bufs=1)
        pK2 = open_pool(name="pK2", bufs=1)
        pENCQ = open_pool(name="pENCQ", bufs=1)

        encs = pENCS.tile([128, MT, S], BF16, tag="encs")
        for mt in range(MT):
            nc.sync.dma_start(
                out=encs[:, mt, :],
                in_=encT.ap().rearrange("(mt p) s -> p mt s", p=128)[:, mt, :],
            )
        encq = pENCQ.tile([128, MT, QL], BF16, tag="encq")
        for mt in range(MT):
            nc.sync.dma_start(
                out=encq[:, mt, :],
                in_=encTq.ap().rearrange("(mt p) s -> p mt s", p=128)[:, mt, :],
            )
        Q2T = pK2.tile([128, MT, QL], BF16, tag="Q2T")
        for nt in range(MT):
            nsl = slice(nt * 128, nt * 128 + 128)
            wq2s = wpool.tile([128, MT, 128], BF16, tag="wq1s")
            nc.sync.dma_start(out=wq2s, in_=wslices("wq2")[:, :, nsl])
            for qb in range(2):
                qs = slice(qb * 512, qb * 512 + 512)
                pp = psp.tile([128, 512], F32, tag="pp")
                for mt in range(MT):
                    nc.tensor.matmul(pp[:, :], wq2s[:, mt, :], encq[:, mt, qs],
                                     start=(mt == 0), stop=(mt == MT - 1))
                nc.scalar.activation(Q2T[:, nt, qs], pp[:, :], AF.Identity,
                                     bias=bc["cq2"][:, nt:nt + 1])
        close_pool("pENCQ")

        attn2 = pAT2.tile([128, MT, QL], BF16, tag="attn2")
        for hp in range(MT):
            ds = slice(hp * 128, hp * 128 + 128)
            wk2s = wpool.tile([128, MT, 128], BF16, tag="wk1s")
            nc.sync.dma_start(out=wk2s, in_=wslices("wk2")[:, :, ds])
            K2h = hpool.tile([128, S], BF16, tag="KTh")
            for sb in range(4):
                ss = slice(sb * 512, sb * 512 + 512)
                pp = psp.tile([128, 512], F32, tag="pp")
                for mt in range(MT):
                    nc.tensor.matmul(pp[:, :], wk2s[:, mt, :], encs[:, mt, ss],
                                     start=(mt == 0), stop=(mt == MT - 1))
                nc.scalar.activation(K2h[:, ss], pp[:, :], AF.Identity,
                                     bias=bc["ck2"][:, hp:hp + 1])
            VH2 = hpool.tile([128, 16, 2, 65], BF16, tag="VH")
            nc.vector.memset(VH2[:, :, :, 64:65], 1.0)
            for t in range(16):
                row0 = V2_ROW0[t // 4] + (t % 4) * 128
                nc.sync.dma_start(
                    out=VH2[:, t, :, 0:64],
                    in_=v2all[row0:row0 + 128,
                              hp * 128:hp * 128 + 128].rearrange(
                        "p (a b) -> p a b", a=2),
                )
            attention(K2h, VH2, Q2T[:, hp, :], attn2, None, hp)

        close_pool("pK2")
        close_pool("pENCS")

        # ---- out-proj 2 + residual + LN2 -> x2 ----
        pX2 = open_pool(name="pX2", bufs=1, side="right")
        pX2B = open_pool(name="pX2B", bufs=1, side="right")
        x2 = pX2.tile([128, MT, QL], F32, tag="x2")
        x2bf = pX2B.tile([128, MT, QL], BF16, tag="x2bf")
        for qb in range(2):
            qs = slice(qb * 512, qb * 512 + 512)
            z2 = lnz.tile([128, MT, 512], F32, tag="z1")
            for nt in range(MT):
                wo2s = wpool.tile([128, MT, 128], BF16, tag="wo1s")
                nc.sync.dma_start(
                    out=wo2s,
                    in_=wslices("wo2")[:, :, nt * 128:nt * 128 + 128],
                )
                pp = psp.tile([128, 512], F32, tag="pp")
                for dt in range(MT):
                    nc.tensor.matmul(pp[:, :], wo2s[:, dt, :], attn2[:, dt, qs],
                                     start=(dt == 0), stop=(dt == MT - 1))
                t1 = lnsc.tile([128, 512], F32, tag="lntmp")
                nc.scalar.activation(t1[:, :], pp[:, :], AF.Identity,
                                     bias=bc["co2"][:, nt:nt + 1])
                nc.vector.tensor_add(z2[:, nt, :], t1[:, :], x1[:, nt, qs])
            ln_block(z2, "g2", "be2", x2, x2bf, qb)

        close_pool("pAT2")
        close_pool("pX1")

        # ================= PHASE C: FFN + LN3 -> out =================
        pF = open_pool(name="pF", bufs=1, side="right")
        pF2 = open_pool(name="pF2", bufs=2)
        for qb in range(2):
            qs = slice(qb * 512, qb * 512 + 512)
            hT = pF.tile([128, FT, 512], BF16, tag="hT")
            for ft in range(FT):
                wf1s = pF2.tile([128, MT, 128], BF16, tag="wf1s")
                nc.sync.dma_start(
                    out=wf1s,
                    in_=wd["wf1"].ap().rearrange("(mt p) f -> p mt f", p=128)[
                        :, :, ft * 128:ft * 128 + 128],
                )
                pp = psp.tile([128, 512], F32, tag="pp")
                for mt in range(MT):
                    nc.tensor.matmul(pp[:, :], wf1s[:, mt, :], x2bf[:, mt, qs],
                                     start=(mt == 0), stop=(mt == MT - 1))
                nc.scalar.activation(hT[:, ft, :], pp[:, :], AF.Relu,
                                     bias=bc["cf1"][:, ft:ft + 1])
            z3 = lnz.tile([128, MT, 512], F32, tag="z1")
            for nt in range(MT):
                wf2s = pF2.tile([128, FT, 128], BF16, tag="wf2s")
                nc.sync.dma_start(
                    out=wf2s,
                    in_=wd["wf2"].ap().rearrange("(ft p) d -> p ft d", p=128)[
                        :, :, nt * 128:nt * 128 + 128],
                )
                pp = psp.tile([128, 512], F32, tag="pp")
                for ft in range(FT):
                    nc.tensor.matmul(pp[:, :], wf2s[:, ft, :], hT[:, ft, :],
                                     start=(ft == 0), stop=(ft == FT - 1))
                t1 = lnsc.tile([128, 512], F32, tag="lntmp")
                nc.scalar.activation(t1[:, :], pp[:, :], AF.Identity,
                                     bias=bc["cf2"][:, nt:nt + 1])
                nc.vector.tensor_add(z3[:, nt, :], t1[:, :], x2[:, nt, qs])
            outsb = pF.tile([128, MT, 512], F32, tag="outsb")
            ln_block(z3, "g3", "be3", outsb, None, 0)
            nc.sync.dma_start(
                out=outT.ap().rearrange("(mt p) q -> p mt q", p=128)[:, :, qs],
                in_=outsb,
            )
        close_pool("pF2")
        close_pool("pF")
        close_pool("pX2B")
        close_pool("pX2")

        for nm in reversed(list(_cms)):
            close_pool(nm)

    return nc


_CACHED = {}


def _get_nc():
    if "nc" not in _CACHED:
        nc = build_nc()
        legalize_waits(nc)
        _CACHED["nc"] = nc
    return _CACHED["nc"]


def _colbias(v, k=8):
    return np.ascontiguousarray(np.asarray(v, np.float32).reshape(k, 128).T)


def _bf(a):
    return np.ascontiguousarray(np.asarray(a)).astype(ml_dtypes.bfloat16)


def _make_mask(j):
    q0s = (0, 1536) if j == 0 else (512, 1024)
    m = np.zeros((2, 8, 128, 512), np.float32)
    for sl in range(2):
        q0 = q0s[sl]
        for ki in range(8):
            kt = ki if sl == 0 else 8 + ki
            k0 = kt * 128
            i = np.arange(128)[:, None]
            jq = np.arange(512)[None, :]
            m[sl, ki] = ((q0 + jq) >= (k0 + i)).astype(np.float32)
    return m.astype(ml_dtypes.bfloat16)


def kernel(**inputs):
    x = np.asarray(inputs["x"], np.float32)
    enc = np.asarray(inputs["encoder_output"], np.float32)
    shared = {}
    for name in ("wq1", "wk1", "wv1", "wo1", "wq2", "wk2", "wv2", "wo2",
                 "wf1", "wf2"):
        shared[name] = _bf(inputs[name])
    for src, dst in (("bq1", "cq1"), ("bk1", "ck1"), ("bo1", "co1"),
                     ("bq2", "cq2"), ("bk2", "ck2"), ("bo2", "co2"),
                     ("g1", "g1"), ("be1", "be1"), ("g2", "g2"), ("be2", "be2"),
                     ("g3", "g3"), ("be3", "be3")):
        shared[dst] = _colbias(inputs[src], 8)
    shared["cf1"] = _colbias(inputs["bf1"], 32)
    shared["cf2"] = _colbias(inputs["bf2"], 8)
    shared["rv1"] = _bf(np.asarray(inputs["bv1"]).reshape(1, D))
    shared["rv2"] = _bf(np.asarray(inputs["bv2"]).reshape(1, D))
    masks = {0: _make_mask(0), 1: _make_mask(1)}

    in_maps = []
    col_list = []
    for c in range(NCORES):
        b, j = c // 2, c % 2
        q0a, q0b = (0, 1536) if j == 0 else (512, 1024)
        cols = np.r_[q0a:q0a + 512, q0b:q0b + 512]
        col_list.append((b, cols))
        xTb = np.ascontiguousarray(x[b].T)
        encTb = np.ascontiguousarray(enc[b].T)
        m = dict(shared)
        m["xT"] = _bf(xTb)
        m["xTq"] = _bf(xTb[:, cols])
        m["xres"] = np.ascontiguousarray(xTb[:, cols])
        m["encT"] = _bf(encTb)
        m["encTq"] = _bf(encTb[:, cols])
        m["bigmask"] = masks[j]
        in_maps.append(m)

    global _LAST_IN_MAPS
    _LAST_IN_MAPS = in_maps
    nc = _get_nc()
    res = run_bass_kernel_spmd(nc, in_maps, core_ids=list(range(NCORES)))
    out = np.empty((B, S, D), np.float32)
    for c in range(NCORES):
        b, cols = col_list[c]
        out[b, cols, :] = res.results[c]["outT"].T
    return out



# revision 2
# speedup vs baseline: 1.0342x; 1.0342x over previous
"""Trainium2 Bass kernel for a post-LN transformer decoder layer (v2).

Sharding: 8 cores = 4 batches x 2 token-halves (interleaved quarters:
pair-rank j=0 handles global rows [0:512)+[1536:2048), j=1 [512:1536)).
Optimized vs v1:
  - host-pretiled weights: every weight-tile DMA is >=2KB contiguous per
    partition line (v1 DMA'd 256B fragments and stalled the PE)
  - all projections precomputed all-heads at N=512 with 4-deep PSUM
    pipelining (no 128-col matmuls, no bias/broadcast matmuls)
  - V bias folded into the out-proj bias on host (bo' = bo + bv @ wo);
    K bias dropped (softmax-invariant)
  - attention runs 4 heads interleaved, software-pipelined one k-tile
    deep (AV of kt-1 issued after scores of kt) using all 8 PSUM banks,
    so the PE never waits on the Scalar engine's exp and the HAM clock
    gate stays warm
  - phase boundaries overlapped (LN stat chains hide under next-phase
    matmuls; the V2 AllGather hides under Q2/K2 projections)
"""

import sys

sys.path.insert(0, "/opt/trn_rl_repo")

import numpy as np
import ml_dtypes

import concourse.bass as bass
import concourse.tile as tile
from concourse import mybir
from concourse.bass_utils import run_bass_kernel_spmd

BF16 = mybir.dt.bfloat16
F32 = mybir.dt.float32
AF = mybir.ActivationFunctionType

D = 1024       # d_model
DFF = 4096
B, S = 4, 2048
NCORES = 8
QL = 1024      # local query rows per core
EPS = 1e-6
MT = 8         # d_model 128-tiles
FT = 32        # d_ff 128-tiles
SCALE = 0.125  # 1/sqrt(head_dim)
SLOT_NKT = (8, 16)  # uniform k-tile count per query slot; masks do the rest

# global key-tile t -> (pair_rank, local st) for the V2 AllGather result:
# rank j=0 owns quarters Q0,Q3 (local st 0-3 = tiles 0-3, st 4-7 = 12-15),
# rank j=1 owns Q1,Q2 (local st 0-3 = tiles 4-7, st 4-7 = tiles 8-11).
V2_SRC = ([(0, t) for t in range(4)] +
          [(1, t - 4) for t in range(4, 12)] +
          [(0, t - 8) for t in range(12, 16)])


def legalize_waits(nc, max_waits=1):
    """This walrus build accepts at most one sync-wait per instruction.
    Hoist excess waits onto same-engine NoOps inserted just before."""
    nid = 0
    for fn in nc.m.functions:
        for bb in fn.blocks:
            new = []
            changed = False
            for inst in bb.instructions:
                si = inst.sync_info
                if si is not None and si.on_wait and len(si.on_wait) > max_waits:
                    waits = list(si.on_wait)
                    for w in waits[:-max_waits]:
                        nid += 1
                        nop = mybir.InstNoOp(name=f"I-waitfix-{nid}", ins=[], outs=[])
                        nop.engine = inst.engine
                        nop.sync_info = mybir.SyncInfo(on_wait=[w], on_update=[])
                        new.append(nop)
                    inst.sync_info = mybir.SyncInfo(
                        on_wait=waits[-max_waits:], on_update=list(si.on_update)
                    )
                    changed = True
                new.append(inst)
            if changed:
                bb.instructions = new


def build_nc():
    nc = bass.Bass(num_devices=NCORES)

    # -------- DRAM tensors (all host-pretiled for contiguous DMA) --------
    xT = nc.dram_tensor("xT", [128, MT, S], BF16, kind="ExternalInput")
    xTq = nc.dram_tensor("xTq", [128, MT, QL], BF16, kind="ExternalInput")
    xres = nc.dram_tensor("xres", [128, MT, QL], F32, kind="ExternalInput")
    encT = nc.dram_tensor("encT", [128, MT, S], BF16, kind="ExternalInput")
    encq = nc.dram_tensor("encq", [128, MT, QL], BF16, kind="ExternalInput")
    wd = {}
    # [p, out_tile, in_tile, 128] : slice [:, ot, :, :] is 2KB/partition
    for name in ("wq1", "wk1", "wo1", "wq2", "wk2", "wo2"):
        wd[name] = nc.dram_tensor(name, [128, MT, MT, 128], BF16,
                                  kind="ExternalInput")
    wd["wf1"] = nc.dram_tensor("wf1", [128, FT, MT, 128], BF16,
                               kind="ExternalInput")
    wd["wf2"] = nc.dram_tensor("wf2", [128, MT, FT, 128], BF16,
                               kind="ExternalInput")
    # wv1/wv2 loaded whole: [p, mt, 1024]
    wd["wv1"] = nc.dram_tensor("wv1", [128, MT, D], BF16, kind="ExternalInput")
    wd["wv2"] = nc.dram_tensor("wv2", [128, MT, D], BF16, kind="ExternalInput")
    bias_specs = (
        ("cq1", 8), ("co1", 8), ("cq2", 8), ("co2", 8),
        ("cf1", 32), ("cf2", 8),
        ("g1", 8), ("be1", 8), ("g2", 8), ("be2", 8), ("g3", 8), ("be3", 8),
    )
    bcd = {}
    for name, k in bias_specs:
        bcd[name] = nc.dram_tensor(name, [128, k], F32, kind="ExternalInput")
    sel2d = nc.dram_tensor("sel2", [2, 128], BF16, kind="ExternalInput")
    bigmask = nc.dram_tensor("bigmask", [128, 2, 8, 512], BF16,
                             kind="ExternalInput")
    outT = nc.dram_tensor("outT", [128, MT, QL], F32, kind="ExternalOutput")
    v2locH = [nc.dram_tensor(f"v2loc{i}", [128, 4 * D], BF16) for i in (0, 1)]
    v2allH = [nc.dram_tensor(f"v2all{i}", [256, 4 * D], BF16) for i in (0, 1)]

    with tile.TileContext(nc) as tc:
        _cms = {}

        def open_pool(**kw):
            cm = tc.tile_pool(**kw)
            _cms[kw["name"]] = cm
            return cm.__enter__()

        def close_pool(pool_name):
            _cms.pop(pool_name).__exit__(None, None, None)

        # ---- global pools ----
        const = open_pool(name="const", bufs=1)
        wp8 = open_pool(name="wp8", bufs=3)      # [128,8,128] weight tiles
        wpv = open_pool(name="wpv", bufs=1)      # [128,8,1024] wv tiles
        epool = open_pool(name="epool", bufs=6)  # exp tiles
        usbp = open_pool(name="usbp", bufs=4)    # u psum->sbuf copies
        prb = open_pool(name="prb", bufs=4)      # recip rows (bf16)
        sp = open_pool(name="sp", bufs=2)        # small scratch rows
        psp = open_pool(name="psp", bufs=2, space="PSUM")  # s:2x2 + u:4 banks

        bc = {}
        for name, k in bias_specs:
            t = const.tile([128, k], F32, tag=name)
            nc.sync.dma_start(out=t, in_=bcd[name][:, :])
            bc[name] = t
        sel2 = const.tile([2, 128], BF16, tag="sel2")
        nc.sync.dma_start(out=sel2, in_=sel2d[:, :])
        ones_col = const.tile([128, 1], BF16, tag="ones_col")
        nc.vector.memset(ones_col, 1.0)
        ones_row = const.tile([1, 128], BF16, tag="ones_row")
        nc.vector.memset(ones_row, 1.0)
        epsr = const.tile([1, 1], F32, tag="epsr")
        nc.vector.memset(epsr, EPS)
        pQV = open_pool(name="pQV", bufs=1)
        QTzG = pQV.tile([128, 16, QL], BF16, tag="QTzG")
        nc.vector.memset(QTzG, 0.0)
        VHG = pQV.tile([128, 16, 16, 65], BF16, tag="VHG")
        nc.vector.memset(VHG[:, :, :, 64:65], 1.0)

        # ---------------- helpers ----------------
        def ln_stats(z):
            """LayerNorm stats over the partition(d_model) axis of z
            [128, MT, 512] f32.  Returns (mubf, rsbf) [1,512] bf16."""
            psum_s = psp.tile([65, 512], F32, tag="u", bufs=4)
            psum_q = psp.tile([65, 512], F32, tag="u", bufs=4)
            for mt in range(MT):
                zbf = sp.tile([128, 512], BF16, tag="zbf")
                nc.scalar.activation(zbf[:, :], z[:, mt, :], AF.Copy)
                zsq = sp.tile([128, 512], BF16, tag="zsq")
                nc.vector.tensor_mul(zsq[:, :], zbf[:, :], zbf[:, :])
                nc.tensor.matmul(psum_s[0:1, :], ones_col[:, :], zbf[:, :],
                                 start=(mt == 0), stop=(mt == MT - 1))
                nc.tensor.matmul(psum_q[0:1, :], ones_col[:, :], zsq[:, :],
                                 start=(mt == 0), stop=(mt == MT - 1))
            mu = sp.tile([1, 512], F32, tag="row_mu", bufs=1)
            nc.scalar.activation(mu[:, :], psum_s[0:1, :], AF.Copy, scale=1.0 / D)
            t = sp.tile([1, 512], F32, tag="row_t", bufs=1)
            nc.scalar.activation(t[:, :], psum_q[0:1, :], AF.Copy, scale=1.0 / D)
            musq = sp.tile([1, 512], F32, tag="recq")
            nc.vector.tensor_mul(musq[:, :], mu[:, :], mu[:, :])
            nc.vector.tensor_sub(t[:, :], t[:, :], musq[:, :])
            nc.scalar.activation(t[:, :], t[:, :], AF.Sqrt, bias=epsr[:, :])
            rstd = sp.tile([1, 512], F32, tag="recq")
            nc.vector.reciprocal(rstd[:, :], t[:, :])
            mubf = sp.tile([1, 512], BF16, tag="row_mubf")
            nc.scalar.activation(mubf[:, :], mu[:, :], AF.Copy)
            rsbf = sp.tile([1, 512], BF16, tag="row_rsbf")
            nc.scalar.activation(rsbf[:, :], rstd[:, :], AF.Copy)
            return mubf, rsbf

        def ln_apply(z, stats, gname, bename, xout, xbfout, qb):
            """Apply LN given stats; write xout[:, :, qb*512:+512] f32
            (+ optional bf16 copy)."""
            mubf, rsbf = stats
            mu_b = psp.tile([128, 512], F32, tag="u", bufs=4)
            nc.tensor.matmul(mu_b[:, :], ones_row[:, :], mubf[:, :],
                             start=True, stop=True)
            rs_b = psp.tile([128, 512], F32, tag="u", bufs=4)
            nc.tensor.matmul(rs_b[:, :], ones_row[:, :], rsbf[:, :],
                             start=True, stop=True)
            qs = slice(qb * 512, qb * 512 + 512)
            g = bc[gname]
            be = bc[bename]
            for mt in range(MT):
                tmp = sp.tile([128, 512], F32, tag="lntmp")
                nc.vector.tensor_sub(tmp[:, :], z[:, mt, :], mu_b[:, :])
                nc.vector.tensor_mul(tmp[:, :], tmp[:, :], rs_b[:, :])
                nc.vector.tensor_scalar(
                    xout[:, mt, qs], tmp[:, :],
                    g[:, mt:mt + 1], be[:, mt:mt + 1],
                    op0=mybir.AluOpType.mult, op1=mybir.AluOpType.add,
                )
                if xbfout is not None:
                    nc.scalar.activation(xbfout[:, mt, qs], xout[:, mt, qs],
                                         AF.Copy)

        def proj_q(QTz, hp, wtile, src, bias, src_cols, qs):
            """Q projection written into the per-head zero-padded layout:
            head 2hp keeps rows 0-63, head 2hp+1 rows 64-127."""
            pp = psp.tile([128, 512], F32, tag="s")
            for mt in range(MT):
                nc.tensor.matmul(pp[:, :], wtile[:, mt, :], src[:, mt, src_cols],
                                 start=(mt == 0), stop=(mt == MT - 1))
            nc.scalar.activation(QTz[0:64, 2 * hp, qs], pp[0:64, :],
                                 AF.Identity, bias=bias[0:64, hp:hp + 1])
            nc.scalar.activation(QTz[64:128, 2 * hp + 1, qs], pp[64:128, :],
                                 AF.Identity, bias=bias[64:128, hp:hp + 1])

        def proj_to(dst, wtile, src, bias_col, dst_cols, src_cols):
            """dst[:, dst_cols] = sum_mt wtile[:,mt,:].T @ src[:,mt,src_cols]
            (+ bias).  One N=512 psum group."""
            pp = psp.tile([128, 512], F32, tag="s")
            for mt in range(MT):
                nc.tensor.matmul(pp[:, :], wtile[:, mt, :], src[:, mt, src_cols],
                                 start=(mt == 0), stop=(mt == MT - 1))
            if bias_col is None:
                nc.scalar.activation(dst[:, dst_cols], pp[:, :], AF.Copy)
            else:
                nc.scalar.activation(dst[:, dst_cols], pp[:, :], AF.Identity,
                                     bias=bias_col)

        def attention_phase(KT, VH, QTz, attnout, use_mask, maskt, nkts):
            """4-head interleaved attention, one-k-tile software pipeline.
            KT [128, MT, S], VH [128, 16, 16, 65], QTz [128, 16, QL]
            (per-head zero-padded so scores use the full 128-row array),
            attnout [128, MT, QL] bf16.  The normalize chain for a slot is
            spread one-head-per-kt-group across the next slot."""
            deferred = []   # per-head normalize closures from previous slot

            def flush_one():
                if deferred:
                    deferred.pop(0)()

            for quad in range(4):
                heads = [4 * quad + i for i in range(4)]
                for slot in range(2):
                    nkt = nkts[slot]
                    qs = slice(slot * 512, slot * 512 + 512)
                    us = {}
                    for h in heads:
                        us[h] = psp.tile([65, 512], F32, tag="u",
                                         name=f"u{h % 4}", bufs=4)
                    pend = []
                    for kt in range(nkt):
                        cur = []
                        for pi in range(2):
                            h0, h1 = heads[2 * pi], heads[2 * pi + 1]
                            hp = h0 // 2
                            ks = slice(kt * 128, kt * 128 + 128)
                            sblk = psp.tile([128, 1024], F32, tag="s",
                                            name="sblk")
                            nc.tensor.matmul(sblk[:, 0:512], KT[:, hp, ks],
                                             QTz[:, h0, qs], start=True,
                                             stop=True)
                            nc.tensor.matmul(sblk[:, 512:1024], KT[:, hp, ks],
                                             QTz[:, h1, qs], start=True,
                                             stop=True)
                            eblk = epool.tile([128, 1024], BF16, tag="e",
                                              name="eblk", bufs=3)
                            nc.scalar.activation(eblk[:, :], sblk[:, :],
                                                 AF.Exp, scale=SCALE)
                            if use_mask and kt >= nkt - 8:
                                ki = kt - (nkt - 8)
                                nc.vector.tensor_mul(
                                    eblk[:, 0:512], eblk[:, 0:512],
                                    maskt[:, slot, ki, :])
                                nc.vector.tensor_mul(
                                    eblk[:, 512:1024], eblk[:, 512:1024],
                                    maskt[:, slot, ki, :])
                            cur.append((h0, eblk[:, 0:512], kt))
                            cur.append((h1, eblk[:, 512:1024], kt))
                        if kt >= 1:
                            flush_one()
                        for (h, ep, ktp) in pend:
                            nc.tensor.matmul(us[h][:, :], VH[:, ktp, h, :],
                                             ep, start=(ktp == 0),
                                             stop=(ktp == nkt - 1))
                        pend = cur
                    for (h, ep, ktp) in pend:
                        nc.tensor.matmul(us[h][:, :], VH[:, ktp, h, :],
                                         ep, start=(ktp == 0),
                                         stop=(ktp == nkt - 1))

                    # free the u banks right away (DVE copies); the rest of
                    # the normalize is deferred one head per kt-group
                    for h in heads:
                        hp, hs0 = h // 2, (h % 2) * 64
                        ub = usbp.tile([65, 512], F32, tag="usb", bufs=4)
                        nc.scalar.activation(ub[:, :], us[h][:, :], AF.Copy)

                        def mk_fin(ub=ub, hp=hp, hs0=hs0, qs=qs):
                            def fin():
                                rec = sp.tile([1, 512], F32, tag="recq")
                                nc.vector.reciprocal(rec[:, :], ub[64:65, :])
                                recb = prb.tile([1, 512], BF16, tag="recb")
                                nc.vector.tensor_copy(recb[:, :], rec[:, :])
                                rb = psp.tile([64, 512], F32, tag="s")
                                nc.tensor.matmul(rb[:, :], ones_row[:, 0:64],
                                                 recb[:, :], start=True,
                                                 stop=True)
                                nc.vector.tensor_mul(
                                    attnout[hs0:hs0 + 64, hp, qs],
                                    ub[0:64, :], rb[:, :])
                            return fin
                        deferred.append(mk_fin())
            while deferred:
                flush_one()

        # ================= PHASE A =================
        pRES = open_pool(name="pRES", bufs=1, side="right")  # attn1
        attn1 = pRES.tile([128, MT, QL], BF16, tag="attn1")
        pKQV = open_pool(name="pKQV", bufs=1)
        KTall = pKQV.tile([128, MT, S], BF16, tag="KTall")
        pA = open_pool(name="pA", bufs=1)
        xTs = pA.tile([128, MT, S], BF16, tag="xTs")
        for mt in range(MT):
            nc.sync.dma_start(out=xTs[:, mt, :], in_=xT.ap()[:, mt, :])

        # K1: all head-pairs, full S, no bias (softmax-invariant)
        for hp in range(MT):
            wt = wp8.tile([128, MT, 128], BF16, tag="w8")
            nc.sync.dma_start(out=wt, in_=wd["wk1"].ap()[:, hp, :, :])
            for sb in range(4):
                ss = slice(sb * 512, sb * 512 + 512)
                proj_to(KTall[:, hp, :], wt, xTs, None, ss, ss)
        # V1: all heads at once -> VHall [st, h, 65] interleaved layout
        wv1s = wpv.tile([128, MT, D], BF16, tag="wv")
        nc.sync.dma_start(out=wv1s, in_=wd["wv1"].ap())
        for st in range(16):
            ts_ = slice(st * 128, st * 128 + 128)
            for vb in range(2):
                dsl = slice(vb * 512, vb * 512 + 512)
                pp = psp.tile([128, 512], F32, tag="s")
                for mt in range(MT):
                    nc.tensor.matmul(pp[:, :], xTs[:, mt, ts_],
                                     wv1s[:, mt, dsl],
                                     start=(mt == 0), stop=(mt == MT - 1))
                nc.vector.tensor_copy(
                    VHG[:, st, vb * 8:vb * 8 + 8, 0:64],
                    pp[:, :].rearrange("p (h d) -> p h d", h=8))
        close_pool("pA")

        # Q1: local query columns, from the per-core xTq input
        pAq = open_pool(name="pAq", bufs=1)
        xqs = pAq.tile([128, MT, QL], BF16, tag="xqs")
        for mt in range(MT):
            nc.sync.dma_start(out=xqs[:, mt, :], in_=xTq.ap()[:, mt, :])
        for hp in range(MT):
            wt = wp8.tile([128, MT, 128], BF16, tag="w8")
            nc.sync.dma_start(out=wt, in_=wd["wq1"].ap()[:, hp, :, :])
            for qb in range(2):
                qs = slice(qb * 512, qb * 512 + 512)
                proj_q(QTzG, hp, wt, xqs, bc["cq1"], qs, qs)
        close_pool("pAq")

        pMSK = open_pool(name="pMSK", bufs=1)
        maskt = pMSK.tile([128, 2, 8, 512], BF16, tag="maskt")
        nc.sync.dma_start(out=maskt, in_=bigmask.ap())

        attention_phase(KTall, VHG, QTzG, attn1, True, maskt, SLOT_NKT)
        close_pool("pMSK")
        close_pool("pKQV")

        # xres loads while the O1 matmuls start (only the DVE adds wait on it)
        pXRES = open_pool(name="pXRES", bufs=1, side="right")
        xres_t = pXRES.tile([128, MT, QL], F32, tag="xres_t")
        for mt in range(MT):
            nc.sync.dma_start(out=xres_t[:, mt, :], in_=xres.ap()[:, mt, :])

        # ---- O1 + residual + LN1 -> x1 ----
        pV2 = open_pool(name="pV2", bufs=2)   # V2 staging (used in phase B)
        pLNZ = open_pool(name="pLNZ", bufs=2)
        zs = {}
        for qb in range(2):
            qs = slice(qb * 512, qb * 512 + 512)
            z1 = pLNZ.tile([128, MT, 512], F32, tag="z")
            for nt in range(MT):
                wt = wp8.tile([128, MT, 128], BF16, tag="w8")
                nc.sync.dma_start(out=wt, in_=wd["wo1"].ap()[:, nt, :, :])
                pp = psp.tile([128, 512], F32, tag="s")
                for dt in range(MT):
                    nc.tensor.matmul(pp[:, :], wt[:, dt, :], attn1[:, dt, qs],
                                     start=(dt == 0), stop=(dt == MT - 1))
                nc.vector.tensor_scalar(
                    z1[:, nt, :], pp[:, :], bc["co1"][:, nt:nt + 1], None,
                    op0=mybir.AluOpType.add)
                nc.vector.tensor_add(z1[:, nt, :], z1[:, nt, :],
                                     xres_t[:, nt, qs])
            zs[qb] = z1
            zs[qb + 2] = ln_stats(z1)
        close_pool("pXRES")
        close_pool("pRES")

        # Q2 first: independent of x1, hides the LN1 serial chains
        pAq2 = open_pool(name="pAq2", bufs=1)
        encqs = pAq2.tile([128, MT, QL], BF16, tag="encqs")
        for mt in range(MT):
            nc.sync.dma_start(out=encqs[:, mt, :], in_=encq.ap()[:, mt, :])
        for hp in range(MT):
            wt = wp8.tile([128, MT, 128], BF16, tag="w8")
            nc.sync.dma_start(out=wt, in_=wd["wq2"].ap()[:, hp, :, :])
            for qb in range(2):
                qs = slice(qb * 512, qb * 512 + 512)
                proj_q(QTzG, hp, wt, encqs, bc["cq2"], qs, qs)
        close_pool("pAq2")

        pX1 = open_pool(name="pX1", bufs=1, side="right")
        x1 = pX1.tile([128, MT, QL], F32, tag="x1")
        pX1B = open_pool(name="pX1B", bufs=1, side="right")
        x1bf = pX1B.tile([128, MT, QL], BF16, tag="x1bf")

        # ================= PHASE B =================
        # V2 = x1 @ wv2 per half, interleaved with the LN1 applies so the
        # PE never waits on a full LN serial chain
        wv2s = wpv.tile([128, MT, D], BF16, tag="wv")
        nc.sync.dma_start(out=wv2s, in_=wd["wv2"].ap())

        def v2_half(i):
            for st in range(4 * i, 4 * i + 4):
                ts_ = slice(st * 128, st * 128 + 128)
                v2sb = pV2.tile([128, D], BF16, tag="v2sb", bufs=1)
                for vb in range(2):
                    dsl = slice(vb * 512, vb * 512 + 512)
                    pp = psp.tile([128, 512], F32, tag="s")
                    for mt in range(MT):
                        nc.tensor.matmul(pp[:, :], x1bf[:, mt, ts_],
                                         wv2s[:, mt, dsl],
                                         start=(mt == 0), stop=(mt == MT - 1))
                    nc.vector.tensor_copy(v2sb[:, dsl], pp[:, :])
                nc.sync.dma_start(
                    out=v2locH[i].ap().rearrange("p (st d) -> p st d",
                                                 st=4)[:, st - 4 * i, :],
                    in_=v2sb)
            nc.gpsimd.collective_compute(
                "AllGather",
                mybir.AluOpType.bypass,
                replica_groups=[[2 * p, 2 * p + 1] for p in range(4)],
                ins=[v2locH[i][:, :]],
                outs=[v2allH[i][:, :]],
            )

        ln_apply(zs[0], zs[2], "g1", "be1", x1, x1bf, 0)
        v2_half(0)
        ln_apply(zs[1], zs[3], "g1", "be1", x1, x1bf, 1)
        close_pool("pLNZ")
        v2_half(1)
        close_pool("pX1B")

        pKQV2 = open_pool(name="pKQV2", bufs=1)
        K2all = pKQV2.tile([128, MT, S], BF16, tag="K2all")
        pB = open_pool(name="pB", bufs=1)
        encs = pB.tile([128, MT, QL], BF16, tag="encs")
        for mt in range(MT):
            nc.sync.dma_start(out=encs[:, mt, :], in_=encT.ap()[:, mt, 0:QL])

        def vh2_build(i):
            # tiles sourced from gather half i (stl in [4i, 4i+4))
            for t in range(16):
                rank, stl = V2_SRC[t]
                if stl // 4 != i:
                    continue
                v2st = pV2.tile([128, D], BF16, tag="v2sb", bufs=1)
                nc.sync.dma_start(
                    out=v2st,
                    in_=v2allH[i][rank * 128:rank * 128 + 128,
                                  (stl - 4 * i) * D:(stl - 4 * i) * D + D])
                nc.vector.tensor_copy(
                    VHG[:, t, :, 0:64],
                    v2st.rearrange("p (h d) -> p h d", h=16))

        # K2 from encoder, two halves of S, VH2 builds interleaved
        for half in range(2):
            if half == 1:
                for mt in range(MT):
                    nc.sync.dma_start(out=encs[:, mt, :],
                                      in_=encT.ap()[:, mt, QL:S])
            vh2_build(half)
            for hp in range(MT):
                wt = wp8.tile([128, MT, 128], BF16, tag="w8")
                nc.sync.dma_start(out=wt, in_=wd["wk2"].ap()[:, hp, :, :])
                for sb in range(2):
                    ss = slice(sb * 512, sb * 512 + 512)
                    gs = slice(half * QL + sb * 512,
                               half * QL + sb * 512 + 512)
                    proj_to(K2all[:, hp, :], wt, encs, None, gs, ss)
        close_pool("pB")
        pAT2 = open_pool(name="pAT2", bufs=1, side="right")
        attn2 = pAT2.tile([128, MT, QL], BF16, tag="attn2")
        attention_phase(K2all, VHG, QTzG, attn2, False, None, (16, 16))
        close_pool("pKQV2")
        close_pool("pV2")
        close_pool("pQV")

        # ---- O2 + residual + LN2 -> x2 ----
        pX2 = open_pool(name="pX2", bufs=1)
        x2 = pX2.tile([128, MT, QL], F32, tag="x2")
        x2bf = pX2.tile([128, MT, QL], BF16, tag="x2bf")
        pLNZ2 = open_pool(name="pLNZ2", bufs=2)
        zs = {}
        for qb in range(2):
            qs = slice(qb * 512, qb * 512 + 512)
            z2 = pLNZ2.tile([128, MT, 512], F32, tag="z")
            for nt in range(MT):
                wt = wp8.tile([128, MT, 128], BF16, tag="w8")
                nc.sync.dma_start(out=wt, in_=wd["wo2"].ap()[:, nt, :, :])
                pp = psp.tile([128, 512], F32, tag="s")
                for dt in range(MT):
                    nc.tensor.matmul(pp[:, :], wt[:, dt, :], attn2[:, dt, qs],
                                     start=(dt == 0), stop=(dt == MT - 1))
                nc.vector.tensor_scalar(
                    z2[:, nt, :], pp[:, :], bc["co2"][:, nt:nt + 1], None,
                    op0=mybir.AluOpType.add)
                nc.vector.tensor_add(z2[:, nt, :], z2[:, nt, :],
                                     x1[:, nt, qs])
            zs[qb] = z2
            zs[qb + 2] = ln_stats(z2)
        close_pool("pAT2")
        close_pool("pX1")

        # ================= PHASE C: FFN + LN3 =================
        # LN2 applies interleave with the FFN matmul stream per qb half
        pF = open_pool(name="pF", bufs=1)
        pF2 = open_pool(name="pF2", bufs=2)

        def ffn_half(qb):
            qs = slice(qb * 512, qb * 512 + 512)
            hT = pF.tile([128, FT, 512], BF16, tag="hT")
            for ft in range(FT):
                wt = wp8.tile([128, MT, 128], BF16, tag="w8")
                nc.sync.dma_start(out=wt, in_=wd["wf1"].ap()[:, ft, :, :])
                pp = psp.tile([128, 512], F32, tag="s")
                for mt in range(MT):
                    nc.tensor.matmul(pp[:, :], wt[:, mt, :], x2bf[:, mt, qs],
                                     start=(mt == 0), stop=(mt == MT - 1))
                nc.scalar.activation(hT[:, ft, :], pp[:, :], AF.Relu,
                                     bias=bc["cf1"][:, ft:ft + 1])
                if qb == 0 and ft == 0:
                    ln_apply(zs[1], zs[3], "g2", "be2", x2, x2bf, 1)
            z3 = pLNZ2.tile([128, MT, 512], F32, tag="z")
            for nt in range(MT):
                wt2 = pF2.tile([128, FT, 128], BF16, tag="wf2t")
                nc.sync.dma_start(out=wt2, in_=wd["wf2"].ap()[:, nt, :, :])
                pp = psp.tile([128, 512], F32, tag="s")
                for ft in range(FT):
                    nc.tensor.matmul(pp[:, :], wt2[:, ft, :], hT[:, ft, :],
                                     start=(ft == 0), stop=(ft == FT - 1))
                nc.vector.tensor_scalar(
                    z3[:, nt, :], pp[:, :], bc["cf2"][:, nt:nt + 1], None,
                    op0=mybir.AluOpType.add)
                nc.vector.tensor_add(z3[:, nt, :], z3[:, nt, :],
                                     x2[:, nt, qs])
            st = ln_stats(z3)
            outsb = pF.tile([128, MT, 512], F32, tag="outsb")
            ln_apply(z3, st, "g3", "be3", outsb, None, 0)
            nc.sync.dma_start(out=outT.ap()[:, :, qs], in_=outsb)

        ln_apply(zs[0], zs[2], "g2", "be2", x2, x2bf, 0)
        ffn_half(0)
        ffn_half(1)
        close_pool("pF2")
        close_pool("pF")
        close_pool("pLNZ2")
        close_pool("pX2")

        for nm in reversed(list(_cms)):
            close_pool(nm)

    return nc


_CACHED = {}


def _get_nc():
    if "nc" not in _CACHED:
        nc = build_nc()
        legalize_waits(nc)
        _CACHED["nc"] = nc
    return _CACHED["nc"]


def _colbias(v, k=8):
    return np.ascontiguousarray(np.asarray(v, np.float32).reshape(k, 128).T)


def _bf(a):
    return np.ascontiguousarray(np.asarray(a)).astype(ml_dtypes.bfloat16)


def _tile_w(w, n_in_t, n_out_t):
    """[n_in_t*128, n_out_t*128] -> [128, n_out_t, n_in_t, 128] bf16."""
    a = np.asarray(w, np.float32).reshape(n_in_t, 128, n_out_t, 128)
    return _bf(a.transpose(1, 2, 0, 3))


def _tile_xT(xb):
    """x [S?, D] -> transposed tiled [128, MT, S]: element (p, mt, s) =
    x[s, mt*128+p]."""
    a = np.ascontiguousarray(np.asarray(xb, np.float32).T)  # [D, S]
    return a.reshape(MT, 128, -1).transpose(1, 0, 2)

def _make_mask(j):
    q0s = (0, 1536) if j == 0 else (512, 1024)
    m = np.zeros((2, 8, 128, 512), np.float32)
    for sl in range(2):
        q0 = q0s[sl]
        for ki in range(8):
            kt = ki if sl == 0 else 8 + ki
            k0 = kt * 128
            i = np.arange(128)[:, None]
            jq = np.arange(512)[None, :]
            m[sl, ki] = ((q0 + jq) >= (k0 + i)).astype(np.float32)
    # -> [128, 2, 8, 512]
    return _bf(m.transpose(2, 0, 1, 3))


def kernel(**inputs):
    x = np.asarray(inputs["x"], np.float32)
    enc = np.asarray(inputs["encoder_output"], np.float32)
    f32 = lambda k: np.asarray(inputs[k], np.float32)
    shared = {}
    for name in ("wq1", "wk1", "wo1", "wq2", "wk2", "wo2"):
        shared[name] = np.ascontiguousarray(_tile_w(inputs[name], MT, MT))
    shared["wf1"] = np.ascontiguousarray(_tile_w(inputs["wf1"], MT, FT))
    shared["wf2"] = np.ascontiguousarray(_tile_w(inputs["wf2"], FT, MT))
    for name in ("wv1", "wv2"):
        a = f32(name).reshape(MT, 128, D)
        shared[name] = _bf(a.transpose(1, 0, 2))
    # fold V bias into out-proj bias: out = wo.T @ (attn + bv) + bo
    co1 = f32("bo1") + f32("bv1") @ f32("wo1")
    co2 = f32("bo2") + f32("bv2") @ f32("wo2")
    shared["co1"] = _colbias(co1, 8)
    shared["co2"] = _colbias(co2, 8)
    for src, dst in (("bq1", "cq1"), ("bq2", "cq2"),
                     ("g1", "g1"), ("be1", "be1"), ("g2", "g2"),
                     ("be2", "be2"), ("g3", "g3"), ("be3", "be3")):
        shared[dst] = _colbias(inputs[src], 8)
    shared["cf1"] = _colbias(inputs["bf1"], 32)
    shared["cf2"] = _colbias(inputs["bf2"], 8)
    sel = np.zeros((2, 128), np.float32)
    sel[0, 0:64] = 1.0
    sel[1, 64:128] = 1.0
    shared["sel2"] = _bf(sel)
    masks = {0: _make_mask(0), 1: _make_mask(1)}

    in_maps = []
    col_list = []
    for c in range(NCORES):
        b, j = c // 2, c % 2
        q0a, q0b = (0, 1536) if j == 0 else (512, 1024)
        cols = np.r_[q0a:q0a + 512, q0b:q0b + 512]
        col_list.append((b, cols))
        xt_t = _tile_xT(x[b])           # [128, MT, S] f32
        enc_t = _tile_xT(enc[b])
        m = dict(shared)
        m["xT"] = _bf(xt_t)
        m["xTq"] = _bf(xt_t[:, :, cols])
        m["xres"] = np.ascontiguousarray(xt_t[:, :, cols])
        m["encT"] = _bf(enc_t)
        m["encq"] = _bf(enc_t[:, :, cols])
        m["bigmask"] = masks[j]
        in_maps.append(m)

    global _LAST_IN_MAPS
    _LAST_IN_MAPS = in_maps
    nc = _get_nc()
    res = run_bass_kernel_spmd(nc, in_maps, core_ids=list(range(NCORES)))
    out = np.empty((B, S, D), np.float32)
    for c in range(NCORES):
        b, cols = col_list[c]
        o = res.results[c]["outT"]        # [128, MT, QL]
        out[b, cols, :] = o.transpose(2, 1, 0).reshape(QL, D)
    return out
